# revision 7
# baseline (speedup 1.0000x reference)
"""MiMo-V2-Flash top-k MoE router on 8 trn2 NeuronCores.

Reference computation (N_GROUP=1, TOPK_GROUP=1 => group machinery is a no-op;
e_score_correction_bias is structurally zeros per the problem spec):
    logits      = hidden @ weight.T                      [T, E]   fp32
    scores      = sigmoid(logits)
    topk_idx    = top_k(scores, 8).indices               [T, 8]
    topk_weight = scores[topk_idx] / sum(scores[topk_idx])
Sigmoid is monotonic and the bias is zero, so top-8 selection on the raw
logits yields the identical index set/order; sigmoid is applied to just the
8 selected logit values per token.

Sharding: tokens across the 8 cores (1024/core). The host feeds each core
its shard pre-transposed so the contraction dim lands on SBUF partitions
with no on-device transposes.

Matmul precision/speed: plain fp32 matmul runs at 1/4 PE rate and fp32r
(12-bit mantissa) is too coarse for exact top-8 selection. Instead the
product is decomposed into three full-rate fp16 matmuls:
    x = xh + xl/2048,  w = wh + wl/2048   (xh=fp16(x), xl=fp16((x-xh)*2048))
    logits ~= xh@wh + (xh@wl + xl@wh)/2048
The residuals are pre-scaled by 2^11 on the host so they stay in fp16
normal range (unscaled w residuals would be fp16-denormal and risk being
flushed by the PE). xh@wh accumulates in a "main" PSUM tile, the two scaled
correction terms share a "corr" PSUM tile, and eviction computes
main + corr * 2^-11. Max abs logit error vs fp64 is ~5e-6 -- the same
order as a plain fp32 matmul -- with zero top-8 index flips measured.
"""

import numpy as np

import concourse.bass as bass
import concourse.bacc as bacc
import concourse.mybir as mybir
import concourse.tile as tile
from concourse.bass_utils import run_bass_kernel_spmd

N_CORES = 8
T_FULL = 8192
H = 4096
E = 256
TOPK = 8
P = 128
T = T_FULL // N_CORES          # 1024 tokens per core
KC = H // P                    # 32 contraction chunks
TC = T // P                    # 8 token chunks per core
WAVE = 4                       # token chunks per PSUM wave (2 banks each)
LO_SCALE = float(2 ** 11)

F16 = mybir.dt.float16
F32 = mybir.dt.float32
U32 = mybir.dt.uint32


def build_nc() -> bass.Bass:
    nc = bacc.Bacc()
    xh_d = nc.declare_dram_parameter("xh", [H, T], F16, False)
    xl_d = nc.declare_dram_parameter("xl", [H, T], F16, False)
    wh_d = nc.declare_dram_parameter("wh", [H, E], F16, False)
    wl_d = nc.declare_dram_parameter("wl", [H, E], F16, False)
    logits_out = nc.declare_dram_parameter("logits", [T, E], F32, True)
    topw_out = nc.declare_dram_parameter("topw", [T, TOPK], F32, True)
    topi_out = nc.declare_dram_parameter("topi", [T, TOPK], U32, True)

    with tile.TileContext(nc) as tc:
        with (
            tc.tile_pool(name="x_pool", bufs=1) as x_pool,
            tc.tile_pool(name="w_pool", bufs=1) as w_pool,
            tc.tile_pool(name="psum_pool", bufs=1, space="PSUM") as psum_pool,
            tc.tile_pool(name="ev_pool", bufs=1) as ev_pool,
        ):
            wh_t, wl_t, xh_t, xl_t = [], [], [], []
            for k in range(KC):
                wh_k = w_pool.tile([P, E], F16, tag=f"wh{k}", name=f"wh{k}")
                nc.sync.dma_start(out=wh_k[:], in_=wh_d[k * P:(k + 1) * P, :])
                wh_t.append(wh_k)
                wl_k = w_pool.tile([P, E], F16, tag=f"wl{k}", name=f"wl{k}")
                nc.sync.dma_start(out=wl_k[:], in_=wl_d[k * P:(k + 1) * P, :])
                wl_t.append(wl_k)
            for k in range(KC):
                xh_k = x_pool.tile([P, T], F16, tag=f"xh{k}", name=f"xh{k}")
                nc.sync.dma_start(out=xh_k[:], in_=xh_d[k * P:(k + 1) * P, :])
                xh_t.append(xh_k)
                xl_k = x_pool.tile([P, T], F16, tag=f"xl{k}", name=f"xl{k}")
                nc.sync.dma_start(out=xl_k[:], in_=xl_d[k * P:(k + 1) * P, :])
                xl_t.append(xl_k)

            # One PSUM bank per token chunk: [:, :E] accumulates the main
            # xh@wh term, [:, E:] the two scaled correction terms.
            psums = []
            for t in range(TC):
                ps = psum_pool.tile([P, 2 * E], F32, tag=f"ps{t}", name=f"ps{t}")
                psums.append(ps)

            # Persistent output staging (no slot reuse -> no WAR waits).
            lg_all = ev_pool.tile([P, TC, E], F32, tag="lg_all", name="lg_all")
            v8_all = ev_pool.tile([P, TC, TOPK], F32, tag="v8_all", name="v8_all")
            i8_all = ev_pool.tile([P, TC, TOPK], U32, tag="i8_all", name="i8_all")
            w8_all = ev_pool.tile([P, TC, TOPK], F32, tag="w8_all", name="w8_all")
            wn_all = ev_pool.tile([P, TC, TOPK], F32, tag="wn_all", name="wn_all")
            s1_all = ev_pool.tile([P, TC], F32, tag="s1_all", name="s1_all")
            r1_all = ev_pool.tile([P, TC], F32, tag="r1_all", name="r1_all")

            for k in range(KC):
                first, last = (k == 0), (k == KC - 1)
                for t in range(TC):
                    ts = slice(t * P, (t + 1) * P)
                    # One accumulation group spans the whole bank: start only
                    # on the bank's first matmul (marks the 2KB zero-region
                    # pending-zero; the corr half's first write auto-zeroes),
                    # stop only on the bank's last matmul.
                    nc.tensor.matmul(
                        psums[t][:, 0:E], xh_t[k][:, ts], wh_t[k][:],
                        start=first, stop=False,
                    )
                    nc.tensor.matmul(
                        psums[t][:, E:2 * E], xh_t[k][:, ts], wl_t[k][:],
                        start=False, stop=False,
                    )
                    nc.tensor.matmul(
                        psums[t][:, E:2 * E], xl_t[k][:, ts], wh_t[k][:],
                        start=False, stop=last,
                    )

            for t in range(TC):
                lgs = lg_all[:, t, :]
                nc.vector.tensor_scalar_mul(
                    lgs, psums[t][:, E:2 * E], 1.0 / LO_SCALE)
                nc.vector.tensor_add(lgs, psums[t][:, 0:E], lgs)
                nc.vector.max(out=v8_all[:, t, :], in_=lgs)
                nc.vector.max_index(
                    out=i8_all[:, t, :], in_max=v8_all[:, t, :], in_values=lgs)
                nc.scalar.activation(
                    w8_all[:, t, :], v8_all[:, t, :],
                    mybir.ActivationFunctionType.Sigmoid,
                )
                nc.vector.tensor_reduce(
                    s1_all[:, t:t + 1], w8_all[:, t, :],
                    axis=mybir.AxisListType.X, op=mybir.AluOpType.add,
                )
                nc.vector.tensor_scalar_add(
                    s1_all[:, t:t + 1], s1_all[:, t:t + 1], 1e-20)
                nc.vector.reciprocal(r1_all[:, t:t + 1], s1_all[:, t:t + 1])
                nc.vector.tensor_scalar_mul(
                    wn_all[:, t, :], w8_all[:, t, :], r1_all[:, t:t + 1])

            # Batched output DMAs on the ACT hwdge ring (inputs use SP ring).
            nc.scalar.dma_start(
                out=logits_out.rearrange("(t p) e -> p t e", p=P), in_=lg_all[:])
            nc.scalar.dma_start(
                out=topi_out.rearrange("(t p) k -> p t k", p=P), in_=i8_all[:])
            nc.scalar.dma_start(
                out=topw_out.rearrange("(t p) k -> p t k", p=P), in_=wn_all[:])

    nc.finalize()
    return nc


def split_fp16(a: np.ndarray):
    """a (fp32) -> (hi, lo) fp16 with lo pre-scaled by 2^11: a ~= hi + lo/2048."""
    hi = a.astype(np.float16)
    lo = ((a - hi.astype(np.float32)) * LO_SCALE).astype(np.float16)
    return hi, lo


_NC_CACHE: list = []


def _get_nc() -> bass.Bass:
    if not _NC_CACHE:
        _NC_CACHE.append(build_nc())
    return _NC_CACHE[0]


def kernel(hidden_states, weight, e_score_correction_bias=None, **run_kwargs):
    hidden = np.asarray(hidden_states, dtype=np.float32)
    w = np.asarray(weight, dtype=np.float32)
    assert hidden.shape == (T_FULL, H), hidden.shape
    assert w.shape == (E, H), w.shape

    wh, wl = split_fp16(np.ascontiguousarray(w.T))
    in_maps = []
    for c in range(N_CORES):
        shard_t = np.ascontiguousarray(hidden[c * T:(c + 1) * T].T)
        xh, xl = split_fp16(shard_t)
        in_maps.append({"xh": xh, "xl": xl, "wh": wh, "wl": wl})

    nc = _get_nc()
    res = run_bass_kernel_spmd(nc, in_maps, core_ids=list(range(N_CORES)), **run_kwargs)

    logits = np.concatenate([r["logits"] for r in res.results], axis=0)
    topw = np.concatenate([r["topw"] for r in res.results], axis=0)
    topi = np.concatenate([r["topi"] for r in res.results], axis=0)
    out = (logits, topw, topi.astype(np.int32))
    kernel.last_results = res
    return out


# revision 8
# speedup vs baseline: 1.2113x; 1.2113x over previous
"""MiMo-V2-Flash top-k MoE router on 8 trn2 NeuronCores.

Reference computation (N_GROUP=1, TOPK_GROUP=1 => group machinery is a no-op;
e_score_correction_bias is structurally zeros per the problem spec):
    logits      = hidden @ weight.T                      [T, E]   fp32
    scores      = sigmoid(logits)
    topk_idx    = top_k(scores, 8).indices               [T, 8]
    topk_weight = scores[topk_idx] / sum(scores[topk_idx])
Sigmoid is monotonic and the bias is zero, so top-8 selection on the raw
logits yields the identical index set/order; sigmoid is applied to just the
8 selected logit values per token.

Sharding: tokens across the 8 cores (1024/core). The host feeds each core
its shard pre-transposed and pre-tiled partition-major ([128, K, N] with the
contraction chunk on partitions) so every DMA is a large fully-contiguous
transfer and no on-device transposes are needed.

Matmul precision/speed: plain fp32 matmul runs at 1/4 PE rate and fp32r
(12-bit mantissa) is too coarse for exact top-8 selection. Instead the
product is decomposed into three full-rate fp16 matmuls:
    x = xh + xl/2048,  w = wh + wl/2048   (xh=fp16(x), xl=fp16((x-xh)*2048))
    logits ~= xh@wh + (xh@wl + xl@wh)/2048
The residuals are pre-scaled by 2^11 on the host so they stay in fp16
normal range (unscaled w residuals would be fp16-denormal and risk being
flushed by the PE). xh@wh accumulates in the first half of a PSUM bank, the
two scaled correction terms in the second half, and eviction computes
main + corr * 2^-11. Max abs logit error vs fp64 is ~5e-6 -- the same
order as a plain fp32 matmul -- with zero top-8 index flips measured.

DMA/PE overlap: inputs are chunked in k-groups and emitted interleaved
(weights for k-range first, then the x chunks of that range) so the first
matmuls unlock after ~1.5MB of traffic; the PE then streams behind the DMA.
x-lo chunks ride the ACT hwdge ring, everything else the SP ring.
"""

import numpy as np

import concourse.bass as bass
import concourse.bacc as bacc
import concourse.mybir as mybir
import concourse.tile as tile
from concourse.bass_utils import run_bass_kernel_spmd

N_CORES = 8
T_FULL = 8192
H = 4096
E = 256
TOPK = 8
P = 128
T = T_FULL // N_CORES          # 1024 tokens per core
KC = H // P                    # 32 contraction chunks
TC = T // P                    # 8 token chunks per core
GX = 2                         # k-chunks per x DMA group
GW = 8                         # k-chunks per w DMA group
NGX = KC // GX                 # 16 x groups
NGW = KC // GW                 # 4 w groups
LO_SCALE = float(2 ** 11)

F16 = mybir.dt.float16
F32 = mybir.dt.float32
U32 = mybir.dt.uint32


def build_nc() -> bass.Bass:
    nc = bacc.Bacc()
    xh_d = nc.declare_dram_parameter("xh", [P, KC * T], F16, False)
    xl_d = nc.declare_dram_parameter("xl", [P, KC * T], F16, False)
    wh_d = nc.declare_dram_parameter("wh", [P, KC * E], F16, False)
    wl_d = nc.declare_dram_parameter("wl", [P, KC * E], F16, False)
    logits_out = nc.declare_dram_parameter("logits", [T, E], F32, True)
    topw_out = nc.declare_dram_parameter("topw", [T, TOPK], F32, True)
    topi_out = nc.declare_dram_parameter("topi", [T, TOPK], U32, True)

    with tile.TileContext(nc) as tc:
        with (
            tc.tile_pool(name="x_pool", bufs=1) as x_pool,
            tc.tile_pool(name="w_pool", bufs=1) as w_pool,
            tc.tile_pool(name="psum_pool", bufs=1, space="PSUM") as psum_pool,
            tc.tile_pool(name="ev_pool", bufs=1) as ev_pool,
        ):
            xh_g = [x_pool.tile([P, GX * T], F16, tag=f"xh{g}", name=f"xh{g}")
                    for g in range(NGX)]
            xl_g = [x_pool.tile([P, GX * T], F16, tag=f"xl{g}", name=f"xl{g}")
                    for g in range(NGX)]
            wh_g = [w_pool.tile([P, GW * E], F16, tag=f"wh{g}", name=f"wh{g}")
                    for g in range(NGW)]
            wl_g = [w_pool.tile([P, GW * E], F16, tag=f"wl{g}", name=f"wl{g}")
                    for g in range(NGW)]

            # Interleaved emission: weights for a k-range, then its x chunks,
            # so the k=0 matmuls unlock as early as possible.
            for wg in range(NGW):
                ws = slice(wg * GW * E, (wg + 1) * GW * E)
                nc.sync.dma_start(out=wh_g[wg][:], in_=wh_d[:, ws])
                nc.sync.dma_start(out=wl_g[wg][:], in_=wl_d[:, ws])
                for xg in range(wg * GW // GX, (wg + 1) * GW // GX):
                    xs = slice(xg * GX * T, (xg + 1) * GX * T)
                    nc.sync.dma_start(out=xh_g[xg][:], in_=xh_d[:, xs])
                    nc.scalar.dma_start(out=xl_g[xg][:], in_=xl_d[:, xs])

            # One PSUM bank per token chunk: [:, :E] accumulates the main
            # xh@wh term, [:, E:] the two scaled correction terms.
            psums = [psum_pool.tile([P, 2 * E], F32, tag=f"ps{t}", name=f"ps{t}")
                     for t in range(TC)]

            # Persistent output staging (no slot reuse -> no WAR waits).
            lg_all = ev_pool.tile([P, TC, E], F32, tag="lg_all", name="lg_all")
            v8_all = ev_pool.tile([P, TC, TOPK], F32, tag="v8_all", name="v8_all")
            i8_all = ev_pool.tile([P, TC, TOPK], U32, tag="i8_all", name="i8_all")
            w8_all = ev_pool.tile([P, TC, TOPK], F32, tag="w8_all", name="w8_all")
            wn_all = ev_pool.tile([P, TC, TOPK], F32, tag="wn_all", name="wn_all")
            s1_all = ev_pool.tile([P, TC], F32, tag="s1_all", name="s1_all")
            r1_all = ev_pool.tile([P, TC], F32, tag="r1_all", name="r1_all")

            def xap(tiles, k, t):
                return tiles[k // GX][:, (k % GX) * T + t * P:
                                      (k % GX) * T + (t + 1) * P]

            def wap(tiles, k):
                return tiles[k // GW][:, (k % GW) * E:(k % GW + 1) * E]

            for k in range(KC):
                first, last = (k == 0), (k == KC - 1)
                for t in range(TC):
                    # One accumulation group spans the whole bank: start only
                    # on the bank's first matmul (marks the 2KB zero-region
                    # pending-zero; the corr half's first write auto-zeroes),
                    # stop only on the bank's last matmul.
                    nc.tensor.matmul(
                        psums[t][:, 0:E], xap(xh_g, k, t), wap(wh_g, k),
                        start=first, stop=False,
                    )
                    nc.tensor.matmul(
                        psums[t][:, E:2 * E], xap(xh_g, k, t), wap(wl_g, k),
                        start=False, stop=False,
                    )
                    nc.tensor.matmul(
                        psums[t][:, E:2 * E], xap(xl_g, k, t), wap(wh_g, k),
                        start=False, stop=last,
                    )

            for t in range(TC):
                lgs = lg_all[:, t, :]
                nc.vector.tensor_scalar_mul(
                    lgs, psums[t][:, E:2 * E], 1.0 / LO_SCALE)
                nc.vector.tensor_add(lgs, psums[t][:, 0:E], lgs)
                nc.vector.max(out=v8_all[:, t, :], in_=lgs)
                nc.vector.max_index(
                    out=i8_all[:, t, :], in_max=v8_all[:, t, :], in_values=lgs)
                nc.scalar.activation(
                    w8_all[:, t, :], v8_all[:, t, :],
                    mybir.ActivationFunctionType.Sigmoid,
                )
                nc.vector.tensor_reduce(
                    s1_all[:, t:t + 1], w8_all[:, t, :],
                    axis=mybir.AxisListType.X, op=mybir.AluOpType.add,
                )
                nc.vector.tensor_scalar_add(
                    s1_all[:, t:t + 1], s1_all[:, t:t + 1], 1e-20)
                nc.vector.reciprocal(r1_all[:, t:t + 1], s1_all[:, t:t + 1])
                nc.vector.tensor_scalar_mul(
                    wn_all[:, t, :], w8_all[:, t, :], r1_all[:, t:t + 1])

            # Batched output DMAs on the ACT hwdge ring.
            nc.scalar.dma_start(
                out=logits_out.rearrange("(t p) e -> p t e", p=P), in_=lg_all[:])
            nc.scalar.dma_start(
                out=topi_out.rearrange("(t p) k -> p t k", p=P), in_=i8_all[:])
            nc.scalar.dma_start(
                out=topw_out.rearrange("(t p) k -> p t k", p=P), in_=wn_all[:])

    nc.finalize()
    return nc


def split_fp16(a: np.ndarray):
    """a (fp32) -> (hi, lo) fp16 with lo pre-scaled by 2^11: a ~= hi + lo/2048."""
    hi = a.astype(np.float16)
    lo = ((a - hi.astype(np.float32)) * LO_SCALE).astype(np.float16)
    return hi, lo


def pack_pm(a2d: np.ndarray) -> np.ndarray:
    """[KC*P, N] -> partition-major [P, KC*N] (rows fully contiguous)."""
    n = a2d.shape[1]
    return np.ascontiguousarray(
        a2d.reshape(KC, P, n).transpose(1, 0, 2).reshape(P, KC * n))


_NC_CACHE: list = []


def _get_nc() -> bass.Bass:
    if not _NC_CACHE:
        _NC_CACHE.append(build_nc())
    return _NC_CACHE[0]


def kernel(hidden_states, weight, e_score_correction_bias=None, **run_kwargs):
    hidden = np.asarray(hidden_states, dtype=np.float32)
    w = np.asarray(weight, dtype=np.float32)
    assert hidden.shape == (T_FULL, H), hidden.shape
    assert w.shape == (E, H), w.shape

    whf, wlf = split_fp16(np.ascontiguousarray(w.T))
    wh_p, wl_p = pack_pm(whf), pack_pm(wlf)
    in_maps = []
    for c in range(N_CORES):
        shard_t = np.ascontiguousarray(hidden[c * T:(c + 1) * T].T)
        xh, xl = split_fp16(shard_t)
        in_maps.append({"xh": pack_pm(xh), "xl": pack_pm(xl),
                        "wh": wh_p, "wl": wl_p})

    nc = _get_nc()
    res = run_bass_kernel_spmd(nc, in_maps, core_ids=list(range(N_CORES)), **run_kwargs)

    logits = np.concatenate([r["logits"] for r in res.results], axis=0)
    topw = np.concatenate([r["topw"] for r in res.results], axis=0)
    topi = np.concatenate([r["topi"] for r in res.results], axis=0)
    out = (logits, topw, topi.astype(np.int32))
    kernel.last_results = res
    return out


# revision 9
# speedup vs baseline: 1.4116x; 1.1654x over previous
"""MiMo-V2-Flash top-k MoE router on 8 trn2 NeuronCores.

Reference computation (N_GROUP=1, TOPK_GROUP=1 => group machinery is a no-op;
e_score_correction_bias is structurally zeros per the problem spec):
    logits      = hidden @ weight.T                      [T, E]   fp32
    scores      = sigmoid(logits)
    topk_idx    = top_k(scores, 8).indices               [T, 8]
    topk_weight = scores[topk_idx] / sum(scores[topk_idx])
Sigmoid is monotonic and the bias is zero, so top-8 selection on the raw
logits yields the identical index set/order; sigmoid is applied to just the
8 selected logit values per token.

Sharding: tokens across the 8 cores (1024/core). The host feeds each core
its shard pre-transposed and pre-tiled partition-major ([128, K, N] with the
contraction chunk on partitions) so every DMA is a large fully-contiguous
transfer and no on-device transposes are needed.

Matmul precision/speed: plain fp32 matmul runs at 1/4 PE rate and fp32r
(12-bit mantissa) is too coarse for exact top-8 selection. Instead the
product is decomposed into three full-rate fp16 matmuls:
    x = xh + xl/2048,  w = wh + wl/2048   (xh=fp16(x), xl=fp16((x-xh)*2048))
    logits ~= xh@wh + (xh@wl + xl@wh)/2048
The residuals are pre-scaled by 2^11 on the host so they stay in fp16
normal range (unscaled w residuals would be fp16-denormal and risk being
flushed by the PE). xh@wh accumulates in the first half of a PSUM bank, the
two scaled correction terms in the second half, and eviction computes
main + corr * 2^-11. Max abs logit error vs fp64 is ~5e-6 -- the same
order as a plain fp32 matmul -- with zero top-8 index flips measured.

Schedule: inputs stream in 0.25MB contiguous chunks, interleaved k-ascending
across both hwdge rings (SP: xh+wh, ACT: xl+wl) so the k=0 matmuls unlock
after ~0.5MB. Matmuls run k-outer over all 8 PSUM banks for k<26 (DMA
overlap), then t-outer for the last k-chunks so banks retire staggered and
each bank's eviction (ACT scale-copy + DVE add/max8/max_index + ACT
sigmoid-with-accum + DVE reciprocal + ACT scale-mul) overlaps the remaining
matmuls.
"""

import numpy as np

import concourse.bass as bass
import concourse.bacc as bacc
import concourse.mybir as mybir
import concourse.tile as tile
from concourse.bass_utils import run_bass_kernel_spmd

N_CORES = 8
T_FULL = 8192
H = 4096
E = 256
TOPK = 8
P = 128
T = T_FULL // N_CORES          # 1024 tokens per core
KC = H // P                    # 32 contraction chunks
TC = T // P                    # 8 token chunks per core
GX = 1                         # k-chunks per x DMA group
GW = 4                         # k-chunks per w DMA group
NGX = KC // GX
NGW = KC // GW
K_PHASE1 = 26                  # k-outer for k<26, then t-outer drain
LO_SCALE = float(2 ** 11)

F16 = mybir.dt.float16
F32 = mybir.dt.float32
U32 = mybir.dt.uint32


def build_nc() -> bass.Bass:
    nc = bacc.Bacc()
    xh_d = nc.declare_dram_parameter("xh", [P, KC * T], F16, False)
    xl_d = nc.declare_dram_parameter("xl", [P, KC * T], F16, False)
    wh_d = nc.declare_dram_parameter("wh", [P, KC * E], F16, False)
    wl_d = nc.declare_dram_parameter("wl", [P, KC * E], F16, False)
    logits_out = nc.declare_dram_parameter("logits", [T, E], F32, True)
    topw_out = nc.declare_dram_parameter("topw", [T, TOPK], F32, True)
    topi_out = nc.declare_dram_parameter("topi", [T, TOPK], U32, True)

    with tile.TileContext(nc) as tc:
        with (
            tc.tile_pool(name="x_pool", bufs=1) as x_pool,
            tc.tile_pool(name="w_pool", bufs=1) as w_pool,
            tc.tile_pool(name="psum_pool", bufs=1, space="PSUM") as psum_pool,
            tc.tile_pool(name="ev_pool", bufs=1) as ev_pool,
        ):
            xh_g = [x_pool.tile([P, GX * T], F16, tag=f"xh{g}", name=f"xh{g}")
                    for g in range(NGX)]
            xl_g = [x_pool.tile([P, GX * T], F16, tag=f"xl{g}", name=f"xl{g}")
                    for g in range(NGX)]
            wh_g = [w_pool.tile([P, GW * E], F16, tag=f"wh{g}", name=f"wh{g}")
                    for g in range(NGW)]
            wl_g = [w_pool.tile([P, GW * E], F16, tag=f"wl{g}", name=f"wl{g}")
                    for g in range(NGW)]

            # k-ascending interleave; xh/wh ride the SP ring, xl/wl the ACT
            # ring. The k=0 matmul's operands are the rings' first transfers.
            for g in range(NGX):
                if g % (GW // GX) == 0:
                    wg = g // (GW // GX)
                    ws = slice(wg * GW * E, (wg + 1) * GW * E)
                    nc.sync.dma_start(out=wh_g[wg][:], in_=wh_d[:, ws])
                    nc.scalar.dma_start(out=wl_g[wg][:], in_=wl_d[:, ws])
                xs = slice(g * GX * T, (g + 1) * GX * T)
                nc.sync.dma_start(out=xh_g[g][:], in_=xh_d[:, xs])
                nc.scalar.dma_start(out=xl_g[g][:], in_=xl_d[:, xs])

            # One PSUM bank per token chunk: [:, :E] accumulates the main
            # xh@wh term, [:, E:] the two scaled correction terms.
            psums = [psum_pool.tile([P, 2 * E], F32, tag=f"ps{t}", name=f"ps{t}")
                     for t in range(TC)]

            # Persistent output staging (no slot reuse -> no WAR waits).
            lg_all = ev_pool.tile([P, TC, E], F32, tag="lg_all", name="lg_all")
            v8_all = ev_pool.tile([P, TC, TOPK], F32, tag="v8_all", name="v8_all")
            i8_all = ev_pool.tile([P, TC, TOPK], U32, tag="i8_all", name="i8_all")
            w8_all = ev_pool.tile([P, TC, TOPK], F32, tag="w8_all", name="w8_all")
            wn_all = ev_pool.tile([P, TC, TOPK], F32, tag="wn_all", name="wn_all")
            s1_all = ev_pool.tile([P, TC], F32, tag="s1_all", name="s1_all")
            r1_all = ev_pool.tile([P, TC], F32, tag="r1_all", name="r1_all")

            def xap(tiles, k, t):
                return tiles[k // GX][:, (k % GX) * T + t * P:
                                      (k % GX) * T + (t + 1) * P]

            def wap(tiles, k):
                return tiles[k // GW][:, (k % GW) * E:(k % GW + 1) * E]

            def emit_matmuls(k, t):
                # One accumulation group spans the whole bank: start only on
                # the bank's first matmul (marks the 2KB zero-region
                # pending-zero; the corr half's first write auto-zeroes),
                # stop only on the bank's last matmul.
                first, last = (k == 0), (k == KC - 1)
                nc.tensor.matmul(
                    psums[t][:, 0:E], xap(xh_g, k, t), wap(wh_g, k),
                    start=first, stop=False,
                )
                nc.tensor.matmul(
                    psums[t][:, E:2 * E], xap(xh_g, k, t), wap(wl_g, k),
                    start=False, stop=False,
                )
                nc.tensor.matmul(
                    psums[t][:, E:2 * E], xap(xl_g, k, t), wap(wh_g, k),
                    start=False, stop=last,
                )

            def evict(t):
                lgs = lg_all[:, t, :]
                # lgs = corr * 2^-11 on ACT, then += main on DVE.
                nc.scalar.activation(
                    lgs, psums[t][:, E:2 * E],
                    mybir.ActivationFunctionType.Copy, scale=1.0 / LO_SCALE,
                )
                nc.vector.tensor_add(lgs, psums[t][:, 0:E], lgs)
                nc.vector.max(out=v8_all[:, t, :], in_=lgs)
                nc.vector.max_index(
                    out=i8_all[:, t, :], in_max=v8_all[:, t, :], in_values=lgs)
                # sigmoid with fused row-sum; the reference's +1e-20 is an
                # exact fp32 no-op for any realistic top-8 sigmoid sum.
                nc.scalar.activation(
                    w8_all[:, t, :], v8_all[:, t, :],
                    mybir.ActivationFunctionType.Sigmoid,
                    accum_out=s1_all[:, t:t + 1],
                )
                nc.vector.reciprocal(r1_all[:, t:t + 1], s1_all[:, t:t + 1])
                nc.scalar.activation(
                    wn_all[:, t, :], w8_all[:, t, :],
                    mybir.ActivationFunctionType.Copy,
                    scale=r1_all[:, t:t + 1],
                )

            for k in range(K_PHASE1):
                for t in range(TC):
                    emit_matmuls(k, t)
            for t in range(TC):
                for k in range(K_PHASE1, KC):
                    emit_matmuls(k, t)
                evict(t)

            # Batched output DMAs on the SP ring (idle by the tail).
            nc.sync.dma_start(
                out=logits_out.rearrange("(t p) e -> p t e", p=P), in_=lg_all[:])
            nc.sync.dma_start(
                out=topi_out.rearrange("(t p) k -> p t k", p=P), in_=i8_all[:])
            nc.sync.dma_start(
                out=topw_out.rearrange("(t p) k -> p t k", p=P), in_=wn_all[:])

    nc.finalize()
    return nc


def split_fp16(a: np.ndarray):
    """a (fp32) -> (hi, lo) fp16 with lo pre-scaled by 2^11: a ~= hi + lo/2048."""
    hi = a.astype(np.float16)
    lo = ((a - hi.astype(np.float32)) * LO_SCALE).astype(np.float16)
    return hi, lo


def pack_pm(a2d: np.ndarray) -> np.ndarray:
    """[KC*P, N] -> partition-major [P, KC*N] (rows fully contiguous)."""
    n = a2d.shape[1]
    return np.ascontiguousarray(
        a2d.reshape(KC, P, n).transpose(1, 0, 2).reshape(P, KC * n))


_NC_CACHE: list = []


def _get_nc() -> bass.Bass:
    if not _NC_CACHE:
        _NC_CACHE.append(build_nc())
    return _NC_CACHE[0]


def kernel(hidden_states, weight, e_score_correction_bias=None, **run_kwargs):
    hidden = np.asarray(hidden_states, dtype=np.float32)
    w = np.asarray(weight, dtype=np.float32)
    assert hidden.shape == (T_FULL, H), hidden.shape
    assert w.shape == (E, H), w.shape

    whf, wlf = split_fp16(np.ascontiguousarray(w.T))
    wh_p, wl_p = pack_pm(whf), pack_pm(wlf)
    in_maps = []
    for c in range(N_CORES):
        shard_t = np.ascontiguousarray(hidden[c * T:(c + 1) * T].T)
        xh, xl = split_fp16(shard_t)
        in_maps.append({"xh": pack_pm(xh), "xl": pack_pm(xl),
                        "wh": wh_p, "wl": wl_p})

    nc = _get_nc()
    res = run_bass_kernel_spmd(nc, in_maps, core_ids=list(range(N_CORES)), **run_kwargs)

    logits = np.concatenate([r["logits"] for r in res.results], axis=0)
    topw = np.concatenate([r["topw"] for r in res.results], axis=0)
    topi = np.concatenate([r["topi"] for r in res.results], axis=0)
    out = (logits, topw, topi.astype(np.int32))
    kernel.last_results = res
    return out
